# revision 10
# baseline (speedup 1.0000x reference)
"""GraphConv classifier (DGL GraphConv x2 + mean-pool + linear) on 8 trn2 NeuronCores.

Strategy (sharding_hint: edge partitioning by dst + replicated weights +
all-reduce for pooling):
  - Nodes are degree-sorted and packed into 128-node dst tiles, row-sharded
    across the 8 cores (interleaved within each global rank block so all
    cores carry the same degree profile).
  - Layer 1: the per-edge source features (x * outdeg^-1/2, bf16) are
    expanded into an edge-ordered table on the HOST with one slot per
    (dst-tile chunk, dst position) such that chunk k holds every dst's k-th
    in-edge ("identity packing"). On device the aggregation is then a plain
    contiguous DMA stream + one DVE tensor_reduce per tile + one PE
    transpose. No gathers, no one-hot matmuls.
  - Layer 2: h1 is produced sharded, AllGather-ed (bf16 halves), then
    per-edge rows are dma_gather-ed (round-robin over the 4 SWDGE queues so
    descriptor generation runs on all four Q7 core pairs). Aggregation per
    chunk is ONE ldweights(one-hot S) + ONE wide matmul accumulating
    agg[dst, 256] in PSUM; the one-hot family for a whole tile is built in
    a single broadcast tensor_tensor is_equal.
  - Degree normalization is folded into per-partition activation scales and
    rank-1 bias matmuls; per-graph sums are AllReduce-d.
"""

import sys

for _p in ("/opt/trn_rl_repo", "/root/.axon_site/_ro/trn_rl_repo"):
    if _p not in sys.path:
        sys.path.append(_p)

import numpy as np
import ml_dtypes

NC = 8  # cores
P = 128  # partitions / tile node count
G_PAD = 128  # graphs (one tile)

_BF16 = ml_dtypes.bfloat16


def _preprocess(x, src, dst, graph_ids, n_classes):
    """Host-side preprocessing: degree-sorted node packing, layer-1
    edge-ordered feature table, layer-2 edge chunk packing, degree scales."""
    n, d1 = x.shape
    src = np.asarray(src, dtype=np.int64)
    dst = np.asarray(dst, dtype=np.int64)
    gid = np.asarray(graph_ids, dtype=np.int64)
    ne = len(src)

    T = -(-n // (P * NC))  # tiles per core (ceil)
    assert T % 2 == 0, "need an even tile count for the A/B half split"
    TH = T // 2
    npad = P * NC * T
    shard = P * T
    halfrows = NC * TH * P  # rows per gather table
    assert halfrows <= 32768, "int16 gather index range exceeded"

    outdeg = np.bincount(src, minlength=n).astype(np.float64)
    indeg = np.bincount(dst, minlength=n).astype(np.float64)
    iso = (1.0 / np.sqrt(np.maximum(outdeg, 1.0))).astype(np.float32)
    isi = (1.0 / np.sqrt(np.maximum(indeg, 1.0))).astype(np.float32)
    sqrtin = (np.sqrt(np.maximum(indeg, 1.0))).astype(np.float32)

    # --- degree-sorted node -> (core, tile, pos) packing --------------------
    # Global rank block r holds the r-th 1024 nodes by descending indegree;
    # core c takes every 8th node of the block, so every core's tile r has
    # the same degree profile (aligned chunk budgets, balanced load).
    order_nodes = np.argsort(-indeg, kind="stable")
    order_pad = np.full(npad, -1, np.int64)
    order_pad[:n] = order_nodes
    blocks = order_pad.reshape(T, NC * P)
    node_core = np.zeros(n, np.int64)
    node_tile = np.zeros(n, np.int64)
    node_pos = np.zeros(n, np.int64)
    perm = np.full(npad, -1, np.int64)  # slot -> original node (-1 pad)
    for t in range(T):
        blk = blocks[t]  # interleave: core c gets blk[c::NC]
        for c in range(NC):
            nodes = blk[c::NC]
            for p_i, v in enumerate(nodes):
                if v < 0:
                    continue
                node_core[v] = c
                node_tile[v] = t
                node_pos[v] = p_i
                perm[c * shard + t * P + p_i] = v

    # permuted per-node vectors [NC, shard]
    def gatherv(vec, fill):
        out = np.full(npad, fill, np.float32)
        m = perm >= 0
        out[m] = vec[perm[m]]
        return out.reshape(NC, shard)

    isi_sh = gatherv(isi, 1.0)
    sqrtin_sh = gatherv(sqrtin, 1.0)
    scl1_sh = gatherv(iso * isi, 1.0)
    gid_sh = gatherv(gid.astype(np.float32), -1.0).astype(_BF16)

    d_core = node_core[dst]
    d_tile = node_tile[dst]
    d_pos = node_pos[dst]

    # --- layer-1 identity-packed edge table ---------------------------------
    # per-tile chunk budget = max indegree among that tile's nodes (any core)
    CH1_t = np.zeros(T, np.int64)
    ideg_pad = np.zeros(npad, np.int64)
    m = perm >= 0
    ideg_pad[m] = indeg[perm[m]].astype(np.int64)
    CH1_t = np.maximum(
        ideg_pad.reshape(NC, T, P).max(axis=(0, 2)), 1
    )
    c1_off = np.zeros(T + 1, np.int64)
    c1_off[1:] = np.cumsum(CH1_t)
    ctot1 = int(c1_off[T])

    # rank of each edge within its dst node
    order_d = np.argsort(dst, kind="stable")
    ds = dst[order_d]
    grp = np.flatnonzero(np.r_[True, ds[1:] != ds[:-1]])
    st = np.zeros(ne, np.int64)
    st[grp] = grp
    st = np.maximum.accumulate(st)
    krank = np.arange(ne) - st  # in-edge rank within dst
    e_src = src[order_d]
    e_dc = d_core[order_d]
    e_dt = d_tile[order_d]
    e_dp = d_pos[order_d]
    chunk1 = c1_off[e_dt] + krank
    xt_rows = (np.asarray(x, np.float32) * iso[:, None]).astype(_BF16)
    me1 = np.zeros((NC, ctot1, P, d1), _BF16)
    me1[e_dc, chunk1, e_dp] = xt_rows[e_src]
    me1 = np.ascontiguousarray(me1.transpose(0, 2, 1, 3).reshape(NC, P, ctot1 * d1))

    # --- layer-2 edge packing (dense chunks + one-hot dl) -------------------
    s_half = (node_tile[src] >= TH).astype(np.int64)
    s_row = (
        node_core[src] * (TH * P)
        + (node_tile[src] - s_half * TH) * P
        + node_pos[src]
    )

    key = (d_core * T + d_tile) * 2 + s_half
    order = np.argsort(key, kind="stable")
    key_s = key[order]
    grp_start = np.flatnonzero(np.r_[True, key_s[1:] != key_s[:-1]])
    starts = np.zeros(len(key_s), dtype=np.int64)
    starts[grp_start] = grp_start
    starts = np.maximum.accumulate(starts)
    rnk = np.arange(len(key_s)) - starts

    counts = np.bincount(key, minlength=NC * T * 2)
    kA = counts[0::2].reshape(NC, T)
    kB = counts[1::2].reshape(NC, T)
    CA_t = np.maximum(1, -(-kA.max(axis=0) // P))  # [T]
    CB_t = np.maximum(1, -(-kB.max(axis=0) // P))
    C_t = CA_t + CB_t
    ctot = int(C_t.sum())

    c_off = np.zeros(T + 1, np.int64)
    c_off[1:] = np.cumsum(C_t)
    idx_arr = np.zeros((NC, ctot, P), dtype=np.int16)
    dl_arr = np.full((NC, ctot, P), -1.0, dtype=np.float32)

    e_core = d_core[order]
    e_tile = d_tile[order]
    e_half = s_half[order]
    e_srow = s_row[order]
    e_dpos = d_pos[order]
    chunk_i = (
        c_off[e_tile]
        + rnk // P
        + np.where(e_half == 1, CA_t[e_tile], 0)
    )
    part_i = rnk % P
    idx_arr[e_core, chunk_i, part_i] = e_srow.astype(np.int16)
    dl_arr[e_core, chunk_i, part_i] = e_dpos.astype(np.float32)

    def wrap_region(a):  # [NC, ck, P] -> [NC, 128, ck*8]
        nc_, ck, _ = a.shape
        flat = a.reshape(nc_, ck * P)
        plane = flat.reshape(nc_, ck * 8, 16).transpose(0, 2, 1)
        return np.tile(plane, (1, 8, 1))

    idx_cols = []
    icol_off = np.zeros(T + 1, np.int64)
    for t in range(T):
        regA = idx_arr[:, c_off[t]:c_off[t] + CA_t[t]]
        regB = idx_arr[:, c_off[t] + CA_t[t]:c_off[t + 1]]
        idx_cols.append(wrap_region(regA))
        idx_cols.append(wrap_region(regB))
        icol_off[t + 1] = icol_off[t] + 8 * (CA_t[t] + CB_t[t])
    idx_wrapped = np.ascontiguousarray(np.concatenate(idx_cols, axis=2))
    dl_f = np.ascontiguousarray(
        dl_arr.transpose(0, 2, 1).astype(_BF16)
    )  # [NC, P, ctot]

    gcounts = np.bincount(gid, minlength=G_PAD).astype(np.float32)
    gcounts = np.maximum(gcounts, 1.0)

    iota = np.broadcast_to(np.arange(P, dtype=np.float32), (P, P)).astype(_BF16)
    ident = np.eye(P, dtype=np.float32).astype(_BF16)

    return dict(
        n=n, d1=d1, T=T, TH=TH, npad=npad, shard=shard, halfrows=halfrows,
        CA_t=CA_t, CB_t=CB_t, C_t=C_t, ctot=ctot, c_off=c_off,
        icol_off=icol_off, CH1_t=CH1_t, c1_off=c1_off, ctot1=ctot1,
        isi_sh=isi_sh, sqrtin_sh=sqrtin_sh,
        scl1_sh=scl1_sh, gid_sh=gid_sh, idx_wrapped=idx_wrapped, dl_f=dl_f,
        me1=me1,
        gcounts=gcounts, iota=np.ascontiguousarray(iota),
        ident=np.ascontiguousarray(ident),
    )


def _build_program(pre, hid, out_dim, n_classes, timing_mode=False, fake_cc=None):
    import concourse.bacc as bacc
    import concourse.tile as tile
    import concourse.mybir as mybir

    dt = mybir.dt
    AF = mybir.ActivationFunctionType
    OP = mybir.AluOpType
    AX = mybir.AxisListType

    T, TH, D1, HID, OUT = pre["T"], pre["TH"], pre["d1"], hid, out_dim
    CA_t, CB_t, C_t = pre["CA_t"], pre["CB_t"], pre["C_t"]
    c_off, icol_off, ctot = pre["c_off"], pre["icol_off"], pre["ctot"]
    CH1_t, c1_off, ctot1 = pre["CH1_t"], pre["c1_off"], pre["ctot1"]
    SHARD, HROWS = pre["shard"], pre["halfrows"]
    HSH = SHARD // 2
    NCLS = n_classes
    CMAX = int(C_t.max())
    CABM = int(max(CA_t.max(), CB_t.max()))
    CH1MAX = int(CH1_t.max())
    assert D1 == P and HID % P == 0 and OUT % P == 0
    SH = HID // P  # hidden-dim slices
    SO = OUT // P

    if fake_cc is None:
        fake_cc = timing_mode
    nc = bacc.Bacc(
        "TRN2", target_bir_lowering=False, debug=False,
        num_devices=1 if timing_mode else NC,
        num_swdge_queues=4,
    )

    # ---- I/O ---------------------------------------------------------------
    def din(name, shape, dtype):
        return nc.dram_tensor(name, list(shape), dtype, kind="ExternalInput")

    scl1_in = din("scl1_sh", (SHARD,), dt.float32)
    isi_in = din("isi_sh", (SHARD,), dt.float32)
    sqrtin_in = din("sqrtin_sh", (SHARD,), dt.float32r)
    gid_in = din("gid_sh", (SHARD,), dt.bfloat16)
    idx_in = din("idx_w", (P, int(icol_off[T])), dt.int16)
    dl_in = din("dl_f", (P, ctot), dt.bfloat16)
    me1_in = din("me1", (P, ctot1 * D1), dt.bfloat16)
    w1_in = din("W1", (D1, HID), dt.float32r)
    b1_in = din("b1", (1, HID), dt.float32r)
    w2_in = din("W2", (HID, OUT), dt.float32r)
    b2_in = din("b2", (1, OUT), dt.float32r)
    wc_in = din("Wc", (OUT, NCLS), dt.float32)
    bc_in = din("bc", (1, NCLS), dt.float32)
    iota_in = din("iota", (P, P), dt.bfloat16)
    ident_in = din("ident", (P, P), dt.bfloat16)
    cnt_in = din("gcnt", (1, G_PAD), dt.float32)
    rcnt_in = din("grcnt", (G_PAD, 1), dt.float32)
    out_t = nc.dram_tensor(
        "out", [G_PAD, NCLS], dt.float32, kind="ExternalOutput"
    )

    h1loc = nc.dram_tensor("h1loc", [SHARD, HID], dt.bfloat16, kind="Internal")
    tbl2a = nc.dram_tensor(
        "tbl2a", [HROWS, HID], dt.bfloat16, kind="Internal", addr_space="Shared"
    )
    tbl2b = nc.dram_tensor(
        "tbl2b", [HROWS, HID], dt.bfloat16, kind="Internal", addr_space="Shared"
    )
    arin = nc.dram_tensor("arin", [P, OUT], dt.float32, kind="Internal")
    arout = nc.dram_tensor(
        "arout", [P, OUT], dt.float32, kind="Internal", addr_space="Shared"
    )

    rg = [list(range(NC))]
    f32r = dt.float32r

    def allgather(src_ap, dst_tensor):
        if fake_cc:
            nc.sync.dma_start(out=dst_tensor[0:HSH, :], in_=src_ap)
        else:
            nc.gpsimd.collective_compute(
                "AllGather",
                OP.bypass,
                replica_groups=rg,
                ins=[src_ap],
                outs=[dst_tensor[:, :]],
            )

    qctr = [0]  # round-robin SWDGE queue assignment

    with tile.TileContext(nc) as tc:
        with (
            tc.tile_pool(name="const", bufs=1) as cpool,
            tc.tile_pool(name="meta", bufs=4) as pmeta,
            tc.tile_pool(name="mg", bufs=5) as pmg,
            tc.tile_pool(name="mg1", bufs=4) as pmg1,
            tc.tile_pool(name="sel", bufs=3) as psel,
            tc.tile_pool(name="aggs", bufs=3) as pagg,
            tc.tile_pool(name="hout", bufs=3) as phout,
            tc.tile_pool(name="fin", bufs=1) as pfin,
            tc.tile_pool(name="psA", bufs=2, space="PSUM") as psA,
            tc.tile_pool(name="psT", bufs=2, space="PSUM") as psT,
            tc.tile_pool(name="psH", bufs=2, space="PSUM") as psH,
            tc.tile_pool(name="psG", bufs=1, space="PSUM") as psG,
        ):
            # ---- constants -------------------------------------------------
            iota_sb = cpool.tile([P, P], dt.bfloat16)
            nc.sync.dma_start(out=iota_sb[:], in_=iota_in[:, :])
            ident_sb = cpool.tile([P, P], dt.bfloat16)
            nc.sync.dma_start(out=ident_sb[:], in_=ident_in[:, :])
            scl1_sb = cpool.tile([P, T], dt.float32)
            nc.sync.dma_start(
                out=scl1_sb[:], in_=scl1_in.ap().rearrange("(t p) -> p t", p=P)
            )
            isi_sb = cpool.tile([P, T], dt.float32)
            nc.sync.dma_start(
                out=isi_sb[:], in_=isi_in.ap().rearrange("(t p) -> p t", p=P)
            )
            gid_sb = cpool.tile([P, T], dt.bfloat16)
            nc.sync.dma_start(
                out=gid_sb[:], in_=gid_in.ap().rearrange("(t p) -> p t", p=P)
            )
            sqrtin_sb = cpool.tile([1, SHARD], f32r)
            nc.sync.dma_start(out=sqrtin_sb[:], in_=sqrtin_in.ap()[None, :])
            w1_sb = cpool.tile([P, HID], f32r)
            nc.sync.dma_start(out=w1_sb[:], in_=w1_in[:, :])
            b1_sb = cpool.tile([1, HID], f32r)
            nc.sync.dma_start(out=b1_sb[:], in_=b1_in[:, :])
            w2_sb = cpool.tile([P, SH * OUT], f32r)  # [128, slice, OUT]
            nc.sync.dma_start(
                out=w2_sb[:],
                in_=w2_in.ap().rearrange("(s k) o -> k s o", k=P),
            )
            b2_sb = cpool.tile([1, OUT], f32r)
            nc.sync.dma_start(out=b2_sb[:], in_=b2_in[:, :])
            wc_sb = cpool.tile([P, SO * NCLS], dt.float32)
            nc.sync.dma_start(
                out=wc_sb[:],
                in_=wc_in.ap().rearrange("(s k) o -> k s o", k=P),
            )
            bc_sb = cpool.tile([1, NCLS], dt.float32)
            nc.sync.dma_start(out=bc_sb[:], in_=bc_in[:, :])
            cnt_sb = cpool.tile([1, G_PAD], dt.float32)
            nc.sync.dma_start(out=cnt_sb[:], in_=cnt_in[:, :])
            rcnt_sb = cpool.tile([G_PAD, 1], dt.float32)
            nc.sync.dma_start(out=rcnt_sb[:], in_=rcnt_in[:, :])

            # ---- phase 1: streamed identity-packed aggregation -------------
            def layer1_tile(t):
                ch = int(CH1_t[t])
                co = int(c1_off[t])
                mg = pmg1.tile(
                    [P, CH1MAX * D1], dt.bfloat16, tag="mg1", name="mg1"
                )
                nc.sync.dma_start(
                    out=mg[:, : ch * D1],
                    in_=me1_in[:, co * D1:(co + ch) * D1],
                )
                agg_f = pagg.tile([P, D1], dt.float32, tag="aggf", name="aggf")
                nc.vector.tensor_reduce(
                    out=agg_f[:, :],
                    in_=mg[:, : ch * D1].rearrange("p (c d) -> p d c", c=ch),
                    axis=AX.X,
                    op=OP.add,
                )
                agg_b = pagg.tile([P, D1], dt.bfloat16, tag="aggb", name="aggb")
                nc.scalar.activation(
                    out=agg_b[:], in_=agg_f[:], func=AF.Copy, scale=1.0
                )
                tp = psT.tile([P, P], dt.bfloat16, tag="tp", name="tp")
                nc.tensor.transpose(
                    out=tp[:], in_=agg_b[:], identity=ident_sb[:, :]
                )
                aggsb = pagg.tile([P, P], f32r, tag="aggsb", name="aggsb")
                nc.scalar.activation(
                    out=aggsb[:], in_=tp[:], func=AF.Copy, scale=1.0
                )
                # dense1
                hps = psH.tile([P, HID], dt.float32, tag="hps", name="hps")
                nc.tensor.matmul(
                    out=hps[:], lhsT=aggsb[:], rhs=w1_sb[:],
                    start=True, stop=False,
                )
                nc.tensor.matmul(
                    out=hps[:],
                    lhsT=sqrtin_sb[0:1, t * P:(t + 1) * P],
                    rhs=b1_sb[:],
                    start=False, stop=True,
                )
                h1s = phout.tile([P, HID], dt.bfloat16, tag="h1s", name="h1s")
                nc.scalar.activation(
                    out=h1s[:], in_=hps[:], func=AF.Lrelu,
                    scale=scl1_sb[:, t:t + 1], alpha=0.01,
                )
                nc.sync.dma_start(out=h1loc[t * P:(t + 1) * P, :], in_=h1s[:])

            for t in range(T):
                layer1_tile(t)
                if t == TH - 1:
                    allgather(h1loc[0:HSH, :], tbl2a)
            allgather(h1loc[HSH:SHARD, :], tbl2b)

            # ---- phase 2: gathered one-hot aggregation + pooling -----------
            hgps_list = [
                psG.tile([P, G_PAD], dt.float32, name=f"hgps{s}")
                for s in range(SO)
            ]

            partials = [
                cpool.tile([P, OUT], dt.bfloat16, name=f"partA{t}")
                for t in range(T)
            ]

            def layer2_tileA(t):
                ca = int(CA_t[t])
                io, co = int(icol_off[t]), int(c_off[t])
                idx_sb = pmeta.tile(
                    [P, CABM * 8], dt.int16, tag="idxA", name="idxA"
                )
                nc.sync.dma_start(
                    out=idx_sb[:, : ca * 8], in_=idx_in[:, io:io + ca * 8]
                )
                dl_sb = pmeta.tile([P, CABM], dt.bfloat16, tag="dlA", name="dlA")
                nc.sync.dma_start(out=dl_sb[:, :ca], in_=dl_in[:, co:co + ca])
                mg = pmg.tile([P, CABM * HID], dt.bfloat16, tag="mgA", name="mgA")
                nc.gpsimd.dma_gather(
                    out_ap=mg[:, : ca * HID].rearrange(
                        "p (c e) -> p c e", e=HID
                    ),
                    in_ap=tbl2a[0:HROWS, :],
                    idxs_ap=idx_sb[:, : ca * 8],
                    num_idxs=ca * P,
                    num_idxs_reg=ca * P,
                    elem_size=HID,
                    single_packet=False,
                    queue_num=qctr[0] % 4,
                )
                qctr[0] += 1
                s_all = psel.tile([P, CABM * P], dt.bfloat16, tag="SA", name="SA")
                nc.vector.tensor_tensor(
                    out=s_all[:, : ca * P].rearrange("p (c j) -> p c j", c=ca),
                    in0=iota_sb[:, :].unsqueeze(1).broadcast_to([P, ca, P]),
                    in1=dl_sb[:, :ca].unsqueeze(2).broadcast_to([P, ca, P]),
                    op=OP.is_equal,
                )
                agg_nm = psA.tile(
                    [P, HID], dt.float32, tag="aggnm", name="aggnm"
                )
                for c in range(ca):
                    nc.tensor.matmul(
                        out=agg_nm[:],
                        lhsT=s_all[:, c * P:(c + 1) * P],
                        rhs=mg[:, c * HID:(c + 1) * HID],
                        start=(c == 0),
                        stop=(c == ca - 1),
                    )
                nc.scalar.activation(
                    out=partials[t][:], in_=agg_nm[:], func=AF.Copy, scale=1.0
                )

            def layer2_tileB(t):
                ca, cb = int(CA_t[t]), int(CB_t[t])
                cc = ca + cb
                io, co = int(icol_off[t]), int(c_off[t])
                idx_sb = pmeta.tile(
                    [P, CABM * 8], dt.int16, tag="idx", name="idx"
                )
                nc.sync.dma_start(
                    out=idx_sb[:, : cb * 8],
                    in_=idx_in[:, io + ca * 8:io + cc * 8],
                )
                dl_sb = pmeta.tile([P, CABM], dt.bfloat16, tag="dl", name="dl")
                nc.sync.dma_start(
                    out=dl_sb[:, :cb], in_=dl_in[:, co + ca:co + cc]
                )
                mg = pmg.tile([P, CABM * HID], dt.bfloat16, tag="mg", name="mg")
                nc.gpsimd.dma_gather(
                    out_ap=mg[:, : cb * HID].rearrange(
                        "p (c e) -> p c e", e=HID
                    ),
                    in_ap=tbl2b[0:HROWS, :],
                    idxs_ap=idx_sb[:, : cb * 8],
                    num_idxs=cb * P,
                    num_idxs_reg=cb * P,
                    elem_size=HID,
                    single_packet=False,
                    queue_num=qctr[0] % 4,
                )
                qctr[0] += 1
                s_all = psel.tile([P, CABM * P], dt.bfloat16, tag="S", name="S")
                nc.vector.tensor_tensor(
                    out=s_all[:, : cb * P].rearrange("p (c j) -> p c j", c=cb),
                    in0=iota_sb[:, :].unsqueeze(1).broadcast_to([P, cb, P]),
                    in1=dl_sb[:, :cb].unsqueeze(2).broadcast_to([P, cb, P]),
                    op=OP.is_equal,
                )
                agg_nm = psA.tile(
                    [P, HID], dt.float32, tag="aggnm", name="aggnm"
                )
                for c in range(cb):
                    nc.tensor.matmul(
                        out=agg_nm[:],
                        lhsT=s_all[:, c * P:(c + 1) * P],
                        rhs=mg[:, c * HID:(c + 1) * HID],
                        start=(c == 0),
                        stop=(c == cb - 1),
                    )
                # combine B psum with the A-half partial
                agg_b = pagg.tile([P, HID], dt.bfloat16, tag="a2b", name="a2b")
                nc.vector.tensor_tensor(
                    out=agg_b[:], in0=agg_nm[:], in1=partials[t][:], op=OP.add
                )
                aggsb = []
                for s in range(SH):
                    tp = psT.tile([P, P], dt.bfloat16, tag="tp", name="tp")
                    nc.tensor.transpose(
                        out=tp[:],
                        in_=agg_b[:, s * P:(s + 1) * P],
                        identity=ident_sb[:, :],
                    )
                    a = pagg.tile([P, P], f32r, tag="aggsb", name="aggsb")
                    nc.scalar.activation(
                        out=a[:], in_=tp[:], func=AF.Copy, scale=1.0
                    )
                    aggsb.append(a)
                # dense2
                hps = psH.tile([P, OUT], dt.float32, tag="hps", name="hps")
                for s in range(SH):
                    nc.tensor.matmul(
                        out=hps[:],
                        lhsT=aggsb[s][:],
                        rhs=w2_sb[:, s * OUT:(s + 1) * OUT],
                        start=(s == 0),
                        stop=False,
                    )
                nc.tensor.matmul(
                    out=hps[:],
                    lhsT=sqrtin_sb[0:1, t * P:(t + 1) * P],
                    rhs=b2_sb[:],
                    start=False, stop=True,
                )
                h2 = phout.tile([P, OUT], dt.bfloat16, tag="h2", name="h2")
                nc.scalar.activation(
                    out=h2[:], in_=hps[:], func=AF.Lrelu,
                    scale=isi_sb[:, t:t + 1], alpha=0.01,
                )
                oh = psel.tile([P, G_PAD], dt.bfloat16, tag="oh", name="oh")
                nc.vector.tensor_tensor(
                    out=oh[:],
                    in0=iota_sb[:, :],
                    in1=gid_sb[:, t:t + 1].broadcast_to([P, G_PAD]),
                    op=OP.is_equal,
                )
                for s in range(SO):
                    nc.tensor.matmul(
                        out=hgps_list[s][:],
                        lhsT=h2[:, s * P:(s + 1) * P],
                        rhs=oh[:],
                        start=(t == 0),
                        stop=(t == T - 1),
                    )

            for t in range(T):
                layer2_tileA(t)
            for t in range(T):
                layer2_tileB(t)

            # ---- pooling finish + classifier -------------------------------
            hg_sb = pfin.tile([P, OUT], dt.float32)
            for s in range(SO):
                nc.vector.tensor_copy(
                    out=hg_sb[:, s * G_PAD:(s + 1) * G_PAD], in_=hgps_list[s][:]
                )
            nc.sync.dma_start(out=arin[:, :], in_=hg_sb[:])
            if fake_cc:
                nc.sync.dma_start(out=arout[:, :], in_=arin[:, :])
            else:
                nc.gpsimd.collective_compute(
                    "AllReduce",
                    OP.add,
                    replica_groups=rg,
                    ins=[arin[:, :]],
                    outs=[arout[:, :]],
                )
            hgr = pfin.tile([P, OUT], dt.float32)
            nc.sync.dma_start(out=hgr[:], in_=arout[:, :])
            ops = psH.tile([P, NCLS], dt.float32, tag="hps", name="ops")
            for s in range(SO):
                nc.tensor.matmul(
                    out=ops[:],
                    lhsT=hgr[:, s * G_PAD:(s + 1) * G_PAD],
                    rhs=wc_sb[:, s * NCLS:(s + 1) * NCLS],
                    start=(s == 0),
                    stop=False,
                )
            nc.tensor.matmul(
                out=ops[:], lhsT=cnt_sb[:], rhs=bc_sb[:], start=False, stop=True
            )
            res_sb = pfin.tile([P, NCLS], dt.float32)
            nc.scalar.activation(
                out=res_sb[:], in_=ops[:], func=AF.Copy, scale=rcnt_sb[:, 0:1]
            )
            nc.sync.dma_start(out=out_t[:, :], in_=res_sb[:])

    nc.compile()
    return nc


def _make_in_maps(pre, weights):
    W1, b1, W2, b2, Wc, bc = weights
    in_maps = []
    for c in range(NC):
        in_maps.append(
            {
                "scl1_sh": np.ascontiguousarray(pre["scl1_sh"][c]),
                "isi_sh": np.ascontiguousarray(pre["isi_sh"][c]),
                "sqrtin_sh": np.ascontiguousarray(pre["sqrtin_sh"][c]),
                "gid_sh": np.ascontiguousarray(pre["gid_sh"][c]),
                "idx_w": np.ascontiguousarray(pre["idx_wrapped"][c]),
                "dl_f": np.ascontiguousarray(pre["dl_f"][c]),
                "me1": np.ascontiguousarray(pre["me1"][c]),
                "W1": np.ascontiguousarray(W1, np.float32),
                "b1": np.ascontiguousarray(b1, np.float32).reshape(1, -1),
                "W2": np.ascontiguousarray(W2, np.float32),
                "b2": np.ascontiguousarray(b2, np.float32).reshape(1, -1),
                "Wc": np.ascontiguousarray(Wc, np.float32),
                "bc": np.ascontiguousarray(bc, np.float32).reshape(1, -1),
                "iota": pre["iota"],
                "ident": pre["ident"],
                "gcnt": pre["gcounts"].reshape(1, -1),
                "grcnt": (1.0 / pre["gcounts"]).reshape(-1, 1),
            }
        )
    return in_maps


def _run(nc, in_maps, trace=False):
    import time

    from concourse.bass_utils import run_bass_kernel_spmd

    last_exc = None
    for attempt in range(3):
        try:
            return run_bass_kernel_spmd(
                nc, in_maps, core_ids=list(range(NC)), trace=trace
            )
        except Exception as e:  # transient device wedges heal on retry
            last_exc = e
            time.sleep(20 * (attempt + 1))
    raise last_exc


def kernel(x, src, dst, graph_ids, W1, b1, W2, b2, Wc, bc, _trace=False,
           _return_results=False):
    x = np.asarray(x, dtype=np.float32)
    n_classes = int(np.asarray(Wc).shape[1])
    pre = _preprocess(x, src, dst, graph_ids, n_classes)
    hid = int(np.asarray(W1).shape[1])
    out_dim = int(np.asarray(W2).shape[1])
    nc = _build_program(pre, hid, out_dim, n_classes)
    weights = (np.asarray(W1), np.asarray(b1), np.asarray(W2),
               np.asarray(b2), np.asarray(Wc), np.asarray(bc))
    in_maps = _make_in_maps(pre, weights)
    res = _run(nc, in_maps, trace=_trace)
    out = res.results[0]["out"][:G_PAD, :n_classes].astype(np.float32)
    if _return_results:
        return out, res
    return out


# revision 11
# speedup vs baseline: 1.2091x; 1.2091x over previous
"""GraphConv classifier (DGL GraphConv x2 + mean-pool + linear) on 8 trn2 NeuronCores.

Strategy (sharding_hint: edge partitioning by dst + replicated weights +
all-reduce for pooling):
  - Nodes are degree-sorted and packed into 128-node dst tiles, row-sharded
    across the 8 cores (interleaved within each global rank block so all
    cores carry the same degree profile).
  - Layer 1: the per-edge source features (x * outdeg^-1/2, bf16) are
    expanded into an edge-ordered table on the HOST with one slot per
    (dst-tile chunk, dst position) such that chunk k holds every dst's k-th
    in-edge ("identity packing"). On device the aggregation is then a plain
    contiguous DMA stream + one DVE tensor_reduce per tile + one PE
    transpose. No gathers, no one-hot matmuls.
  - Layer 2: h1 is produced sharded, AllGather-ed (bf16 halves), then
    per-edge rows are dma_gather-ed (round-robin over the 4 SWDGE queues so
    descriptor generation runs on all four Q7 core pairs). Aggregation per
    chunk is ONE ldweights(one-hot S) + ONE wide matmul accumulating
    agg[dst, 256] in PSUM; the one-hot family for a whole tile is built in
    a single broadcast tensor_tensor is_equal.
  - Degree normalization is folded into per-partition activation scales and
    rank-1 bias matmuls; per-graph sums are AllReduce-d.
"""

import sys

for _p in ("/opt/trn_rl_repo", "/root/.axon_site/_ro/trn_rl_repo"):
    if _p not in sys.path:
        sys.path.append(_p)

import numpy as np
import ml_dtypes

NC = 8  # cores
P = 128  # partitions / tile node count
G_PAD = 128  # graphs (one tile)

_BF16 = ml_dtypes.bfloat16


def _preprocess(x, src, dst, graph_ids, n_classes):
    """Host-side preprocessing: degree-sorted node packing, layer-1
    edge-ordered feature table, layer-2 edge chunk packing, degree scales."""
    n, d1 = x.shape
    src = np.asarray(src, dtype=np.int64)
    dst = np.asarray(dst, dtype=np.int64)
    gid = np.asarray(graph_ids, dtype=np.int64)
    ne = len(src)

    T = -(-n // (P * NC))  # tiles per core (ceil)
    assert T % 2 == 0, "need an even tile count for the A/B half split"
    TH = T // 2
    npad = P * NC * T
    shard = P * T
    halfrows = NC * TH * P  # rows per gather table
    assert halfrows <= 32768, "int16 gather index range exceeded"

    outdeg = np.bincount(src, minlength=n).astype(np.float64)
    indeg = np.bincount(dst, minlength=n).astype(np.float64)
    iso = (1.0 / np.sqrt(np.maximum(outdeg, 1.0))).astype(np.float32)
    isi = (1.0 / np.sqrt(np.maximum(indeg, 1.0))).astype(np.float32)
    sqrtin = (np.sqrt(np.maximum(indeg, 1.0))).astype(np.float32)

    # --- degree-sorted node -> (core, tile, pos) packing --------------------
    # Global rank block r holds the r-th 1024 nodes by descending indegree;
    # core c takes every 8th node of the block, so every core's tile r has
    # the same degree profile (aligned chunk budgets, balanced load).
    order_nodes = np.argsort(-indeg, kind="stable")
    order_pad = np.full(npad, -1, np.int64)
    order_pad[:n] = order_nodes
    blocks = order_pad.reshape(T, NC * P)
    node_core = np.zeros(n, np.int64)
    node_tile = np.zeros(n, np.int64)
    node_pos = np.zeros(n, np.int64)
    perm = np.full(npad, -1, np.int64)  # slot -> original node (-1 pad)
    for t in range(T):
        blk = blocks[t]  # interleave: core c gets blk[c::NC]
        for c in range(NC):
            nodes = blk[c::NC]
            for p_i, v in enumerate(nodes):
                if v < 0:
                    continue
                node_core[v] = c
                node_tile[v] = t
                node_pos[v] = p_i
                perm[c * shard + t * P + p_i] = v

    # permuted per-node vectors [NC, shard]
    def gatherv(vec, fill):
        out = np.full(npad, fill, np.float32)
        m = perm >= 0
        out[m] = vec[perm[m]]
        return out.reshape(NC, shard)

    isi_sh = gatherv(isi, 1.0)
    sqrtin_sh = gatherv(sqrtin, 1.0)
    scl1_sh = gatherv(iso * isi, 1.0)
    gid_sh = gatherv(gid.astype(np.float32), -1.0).astype(_BF16)

    d_core = node_core[dst]
    d_tile = node_tile[dst]
    d_pos = node_pos[dst]

    # --- layer-1 identity-packed edge table ---------------------------------
    # per-tile chunk budget = max indegree among that tile's nodes (any core)
    CH1_t = np.zeros(T, np.int64)
    ideg_pad = np.zeros(npad, np.int64)
    m = perm >= 0
    ideg_pad[m] = indeg[perm[m]].astype(np.int64)
    CH1_t = np.maximum(
        ideg_pad.reshape(NC, T, P).max(axis=(0, 2)), 1
    )
    c1_off = np.zeros(T + 1, np.int64)
    c1_off[1:] = np.cumsum(CH1_t)
    ctot1 = int(c1_off[T])

    # rank of each edge within its dst node
    order_d = np.argsort(dst, kind="stable")
    ds = dst[order_d]
    grp = np.flatnonzero(np.r_[True, ds[1:] != ds[:-1]])
    st = np.zeros(ne, np.int64)
    st[grp] = grp
    st = np.maximum.accumulate(st)
    krank = np.arange(ne) - st  # in-edge rank within dst
    e_src = src[order_d]
    e_dc = d_core[order_d]
    e_dt = d_tile[order_d]
    e_dp = d_pos[order_d]
    chunk1 = c1_off[e_dt] + krank
    xt_rows = (np.asarray(x, np.float32) * iso[:, None]).astype(_BF16)
    me1 = np.zeros((NC, ctot1, P, d1), _BF16)
    me1[e_dc, chunk1, e_dp] = xt_rows[e_src]
    me1 = np.ascontiguousarray(me1.transpose(0, 2, 1, 3).reshape(NC, P, ctot1 * d1))

    # --- layer-2 edge packing (dense chunks + one-hot dl) -------------------
    s_half = (node_tile[src] >= TH).astype(np.int64)
    s_row = (
        node_core[src] * (TH * P)
        + (node_tile[src] - s_half * TH) * P
        + node_pos[src]
    )

    key = (d_core * T + d_tile) * 2 + s_half
    order = np.argsort(key, kind="stable")
    key_s = key[order]
    grp_start = np.flatnonzero(np.r_[True, key_s[1:] != key_s[:-1]])
    starts = np.zeros(len(key_s), dtype=np.int64)
    starts[grp_start] = grp_start
    starts = np.maximum.accumulate(starts)
    rnk = np.arange(len(key_s)) - starts

    counts = np.bincount(key, minlength=NC * T * 2)
    kA = counts[0::2].reshape(NC, T)
    kB = counts[1::2].reshape(NC, T)
    CA_t = np.maximum(1, -(-kA.max(axis=0) // P))  # [T]
    CB_t = np.maximum(1, -(-kB.max(axis=0) // P))
    C_t = CA_t + CB_t
    ctot = int(C_t.sum())

    c_off = np.zeros(T + 1, np.int64)
    c_off[1:] = np.cumsum(C_t)
    idx_arr = np.zeros((NC, ctot, P), dtype=np.int16)
    dl_arr = np.full((NC, ctot, P), -1.0, dtype=np.float32)

    e_core = d_core[order]
    e_tile = d_tile[order]
    e_half = s_half[order]
    e_srow = s_row[order]
    e_dpos = d_pos[order]
    chunk_i = (
        c_off[e_tile]
        + rnk // P
        + np.where(e_half == 1, CA_t[e_tile], 0)
    )
    part_i = rnk % P
    idx_arr[e_core, chunk_i, part_i] = e_srow.astype(np.int16)
    dl_arr[e_core, chunk_i, part_i] = e_dpos.astype(np.float32)

    def wrap_region(a):  # [NC, ck, P] -> [NC, 128, ck*8]
        nc_, ck, _ = a.shape
        flat = a.reshape(nc_, ck * P)
        plane = flat.reshape(nc_, ck * 8, 16).transpose(0, 2, 1)
        return np.tile(plane, (1, 8, 1))

    idx_cols = []
    icol_off = np.zeros(T + 1, np.int64)
    for t in range(T):
        regA = idx_arr[:, c_off[t]:c_off[t] + CA_t[t]]
        regB = idx_arr[:, c_off[t] + CA_t[t]:c_off[t + 1]]
        idx_cols.append(wrap_region(regA))
        idx_cols.append(wrap_region(regB))
        icol_off[t + 1] = icol_off[t] + 8 * (CA_t[t] + CB_t[t])
    idx_wrapped = np.ascontiguousarray(np.concatenate(idx_cols, axis=2))
    dl_f = np.ascontiguousarray(
        dl_arr.transpose(0, 2, 1).astype(_BF16)
    )  # [NC, P, ctot]

    gcounts = np.bincount(gid, minlength=G_PAD).astype(np.float32)
    gcounts = np.maximum(gcounts, 1.0)

    iota = np.broadcast_to(np.arange(P, dtype=np.float32), (P, P)).astype(_BF16)
    ident = np.eye(P, dtype=np.float32).astype(_BF16)

    return dict(
        n=n, d1=d1, T=T, TH=TH, npad=npad, shard=shard, halfrows=halfrows,
        CA_t=CA_t, CB_t=CB_t, C_t=C_t, ctot=ctot, c_off=c_off,
        icol_off=icol_off, CH1_t=CH1_t, c1_off=c1_off, ctot1=ctot1,
        isi_sh=isi_sh, sqrtin_sh=sqrtin_sh,
        scl1_sh=scl1_sh, gid_sh=gid_sh, idx_wrapped=idx_wrapped, dl_f=dl_f,
        me1=me1,
        gcounts=gcounts, iota=np.ascontiguousarray(iota),
        ident=np.ascontiguousarray(ident),
    )


def _build_program(pre, hid, out_dim, n_classes, timing_mode=False, fake_cc=None):
    import concourse.bacc as bacc
    import concourse.tile as tile
    import concourse.mybir as mybir

    dt = mybir.dt
    AF = mybir.ActivationFunctionType
    OP = mybir.AluOpType
    AX = mybir.AxisListType

    T, TH, D1, HID, OUT = pre["T"], pre["TH"], pre["d1"], hid, out_dim
    CA_t, CB_t, C_t = pre["CA_t"], pre["CB_t"], pre["C_t"]
    c_off, icol_off, ctot = pre["c_off"], pre["icol_off"], pre["ctot"]
    CH1_t, c1_off, ctot1 = pre["CH1_t"], pre["c1_off"], pre["ctot1"]
    SHARD, HROWS = pre["shard"], pre["halfrows"]
    HSH = SHARD // 2
    NCLS = n_classes
    CMAX = int(C_t.max())
    CH1MAX = int(CH1_t.max())
    assert D1 == P and HID % P == 0 and OUT % P == 0
    SH = HID // P  # hidden-dim slices
    SO = OUT // P

    if fake_cc is None:
        fake_cc = timing_mode
    nc = bacc.Bacc(
        "TRN2", target_bir_lowering=False, debug=False,
        num_devices=1 if timing_mode else NC,
        num_swdge_queues=4,
    )

    # ---- I/O ---------------------------------------------------------------
    def din(name, shape, dtype):
        return nc.dram_tensor(name, list(shape), dtype, kind="ExternalInput")

    scl1_in = din("scl1_sh", (SHARD,), dt.float32)
    isi_in = din("isi_sh", (SHARD,), dt.float32)
    sqrtin_in = din("sqrtin_sh", (SHARD,), dt.float32r)
    gid_in = din("gid_sh", (SHARD,), dt.bfloat16)
    idx_in = din("idx_w", (P, int(icol_off[T])), dt.int16)
    dl_in = din("dl_f", (P, ctot), dt.bfloat16)
    me1_in = din("me1", (P, ctot1 * D1), dt.bfloat16)
    w1_in = din("W1", (D1, HID), dt.float32r)
    b1_in = din("b1", (1, HID), dt.float32r)
    w2_in = din("W2", (HID, OUT), dt.float32r)
    b2_in = din("b2", (1, OUT), dt.float32r)
    wc_in = din("Wc", (OUT, NCLS), dt.float32)
    bc_in = din("bc", (1, NCLS), dt.float32)
    iota_in = din("iota", (P, P), dt.bfloat16)
    ident_in = din("ident", (P, P), dt.bfloat16)
    cnt_in = din("gcnt", (1, G_PAD), dt.float32)
    rcnt_in = din("grcnt", (G_PAD, 1), dt.float32)
    out_t = nc.dram_tensor(
        "out", [G_PAD, NCLS], dt.float32, kind="ExternalOutput"
    )

    h1loc = nc.dram_tensor("h1loc", [SHARD, HID], dt.bfloat16, kind="Internal")
    tbl2a = nc.dram_tensor(
        "tbl2a", [HROWS, HID], dt.bfloat16, kind="Internal", addr_space="Shared"
    )
    tbl2b = nc.dram_tensor(
        "tbl2b", [HROWS, HID], dt.bfloat16, kind="Internal", addr_space="Shared"
    )
    arin = nc.dram_tensor("arin", [P, OUT], dt.float32, kind="Internal")
    arout = nc.dram_tensor(
        "arout", [P, OUT], dt.float32, kind="Internal", addr_space="Shared"
    )

    rg = [list(range(NC))]
    f32r = dt.float32r

    def allgather(src_ap, dst_tensor):
        if fake_cc:
            nc.sync.dma_start(out=dst_tensor[0:HSH, :], in_=src_ap)
        else:
            nc.gpsimd.collective_compute(
                "AllGather",
                OP.bypass,
                replica_groups=rg,
                ins=[src_ap],
                outs=[dst_tensor[:, :]],
            )

    qctr = [0]  # round-robin SWDGE queue assignment

    with tile.TileContext(nc) as tc:
        with (
            tc.tile_pool(name="const", bufs=1) as cpool,
            tc.tile_pool(name="meta", bufs=4) as pmeta,
            tc.tile_pool(name="mg", bufs=5) as pmg,
            tc.tile_pool(name="mg1", bufs=4) as pmg1,
            tc.tile_pool(name="sel", bufs=3) as psel,
            tc.tile_pool(name="aggs", bufs=3) as pagg,
            tc.tile_pool(name="hout", bufs=3) as phout,
            tc.tile_pool(name="fin", bufs=1) as pfin,
            tc.tile_pool(name="psA", bufs=2, space="PSUM") as psA,
            tc.tile_pool(name="psT", bufs=2, space="PSUM") as psT,
            tc.tile_pool(name="psH", bufs=2, space="PSUM") as psH,
            tc.tile_pool(name="psG", bufs=1, space="PSUM") as psG,
        ):
            # ---- constants -------------------------------------------------
            iota_sb = cpool.tile([P, P], dt.bfloat16)
            nc.sync.dma_start(out=iota_sb[:], in_=iota_in[:, :])
            ident_sb = cpool.tile([P, P], dt.bfloat16)
            nc.sync.dma_start(out=ident_sb[:], in_=ident_in[:, :])
            scl1_sb = cpool.tile([P, T], dt.float32)
            nc.sync.dma_start(
                out=scl1_sb[:], in_=scl1_in.ap().rearrange("(t p) -> p t", p=P)
            )
            isi_sb = cpool.tile([P, T], dt.float32)
            nc.sync.dma_start(
                out=isi_sb[:], in_=isi_in.ap().rearrange("(t p) -> p t", p=P)
            )
            gid_sb = cpool.tile([P, T], dt.bfloat16)
            nc.sync.dma_start(
                out=gid_sb[:], in_=gid_in.ap().rearrange("(t p) -> p t", p=P)
            )
            sqrtin_sb = cpool.tile([1, SHARD], f32r)
            nc.sync.dma_start(out=sqrtin_sb[:], in_=sqrtin_in.ap()[None, :])
            w1_sb = cpool.tile([P, HID], f32r)
            nc.sync.dma_start(out=w1_sb[:], in_=w1_in[:, :])
            b1_sb = cpool.tile([1, HID], f32r)
            nc.sync.dma_start(out=b1_sb[:], in_=b1_in[:, :])
            w2_sb = cpool.tile([P, SH * OUT], f32r)  # [128, slice, OUT]
            nc.sync.dma_start(
                out=w2_sb[:],
                in_=w2_in.ap().rearrange("(s k) o -> k s o", k=P),
            )
            b2_sb = cpool.tile([1, OUT], f32r)
            nc.sync.dma_start(out=b2_sb[:], in_=b2_in[:, :])
            wc_sb = cpool.tile([P, SO * NCLS], dt.float32)
            nc.sync.dma_start(
                out=wc_sb[:],
                in_=wc_in.ap().rearrange("(s k) o -> k s o", k=P),
            )
            bc_sb = cpool.tile([1, NCLS], dt.float32)
            nc.sync.dma_start(out=bc_sb[:], in_=bc_in[:, :])
            cnt_sb = cpool.tile([1, G_PAD], dt.float32)
            nc.sync.dma_start(out=cnt_sb[:], in_=cnt_in[:, :])
            rcnt_sb = cpool.tile([G_PAD, 1], dt.float32)
            nc.sync.dma_start(out=rcnt_sb[:], in_=rcnt_in[:, :])

            # ---- phase 1: streamed identity-packed aggregation -------------
            def layer1_tile(t):
                ch = int(CH1_t[t])
                co = int(c1_off[t])
                mg = pmg1.tile(
                    [P, CH1MAX * D1], dt.bfloat16, tag="mg1", name="mg1"
                )
                nc.sync.dma_start(
                    out=mg[:, : ch * D1],
                    in_=me1_in[:, co * D1:(co + ch) * D1],
                )
                agg_f = pagg.tile([P, D1], dt.float32, tag="aggf", name="aggf")
                nc.vector.tensor_reduce(
                    out=agg_f[:, :],
                    in_=mg[:, : ch * D1].rearrange("p (c d) -> p d c", c=ch),
                    axis=AX.X,
                    op=OP.add,
                )
                agg_b = pagg.tile([P, D1], dt.bfloat16, tag="aggb", name="aggb")
                nc.scalar.activation(
                    out=agg_b[:], in_=agg_f[:], func=AF.Copy, scale=1.0
                )
                tp = psT.tile([P, P], dt.bfloat16, tag="tp", name="tp")
                nc.tensor.transpose(
                    out=tp[:], in_=agg_b[:], identity=ident_sb[:, :]
                )
                aggsb = pagg.tile([P, P], f32r, tag="aggsb", name="aggsb")
                nc.scalar.activation(
                    out=aggsb[:], in_=tp[:], func=AF.Copy, scale=1.0
                )
                # dense1
                hps = psH.tile([P, HID], dt.float32, tag="hps", name="hps")
                nc.tensor.matmul(
                    out=hps[:], lhsT=aggsb[:], rhs=w1_sb[:],
                    start=True, stop=False,
                )
                nc.tensor.matmul(
                    out=hps[:],
                    lhsT=sqrtin_sb[0:1, t * P:(t + 1) * P],
                    rhs=b1_sb[:],
                    start=False, stop=True,
                )
                h1s = phout.tile([P, HID], dt.bfloat16, tag="h1s", name="h1s")
                nc.scalar.activation(
                    out=h1s[:], in_=hps[:], func=AF.Lrelu,
                    scale=scl1_sb[:, t:t + 1], alpha=0.01,
                )
                nc.sync.dma_start(out=h1loc[t * P:(t + 1) * P, :], in_=h1s[:])

            for t in range(T):
                layer1_tile(t)
                if t == TH - 1:
                    allgather(h1loc[0:HSH, :], tbl2a)
            allgather(h1loc[HSH:SHARD, :], tbl2b)

            # ---- phase 2: gathered one-hot aggregation + pooling -----------
            hgps_list = [
                psG.tile([P, G_PAD], dt.float32, name=f"hgps{s}")
                for s in range(SO)
            ]

            def layer2_tile(t):
                ca, cb = int(CA_t[t]), int(CB_t[t])
                cc = ca + cb
                io, co = int(icol_off[t]), int(c_off[t])
                idx_sb = pmeta.tile([P, CMAX * 8], dt.int16, tag="idx", name="idx")
                nc.sync.dma_start(
                    out=idx_sb[:, : cc * 8], in_=idx_in[:, io:io + cc * 8]
                )
                dl_sb = pmeta.tile([P, CMAX], dt.bfloat16, tag="dl", name="dl")
                nc.sync.dma_start(out=dl_sb[:, :cc], in_=dl_in[:, co:co + cc])
                mg = pmg.tile([P, CMAX * HID], dt.bfloat16, tag="mg", name="mg")
                # A/B half-table gathers round-robin across SWDGE queues
                for tbl, lo, hi in ((tbl2a, 0, ca), (tbl2b, ca, cc)):
                    nc.gpsimd.dma_gather(
                        out_ap=mg[:, lo * HID:hi * HID].rearrange(
                            "p (c e) -> p c e", e=HID
                        ),
                        in_ap=tbl[0:HROWS, :],
                        idxs_ap=idx_sb[:, lo * 8:hi * 8],
                        num_idxs=(hi - lo) * P,
                        num_idxs_reg=(hi - lo) * P,
                        elem_size=HID,
                        single_packet=False,
                        queue_num=qctr[0] % 4,
                    )
                    qctr[0] += 1
                s_all = psel.tile([P, CMAX * P], dt.bfloat16, tag="S", name="S")
                nc.vector.tensor_tensor(
                    out=s_all[:, : cc * P].rearrange("p (c j) -> p c j", c=cc),
                    in0=iota_sb[:, :].unsqueeze(1).broadcast_to([P, cc, P]),
                    in1=dl_sb[:, :cc].unsqueeze(2).broadcast_to([P, cc, P]),
                    op=OP.is_equal,
                )
                agg_nm = psA.tile(
                    [P, HID], dt.float32, tag="aggnm", name="aggnm"
                )
                for c in range(cc):
                    nc.tensor.matmul(
                        out=agg_nm[:],
                        lhsT=s_all[:, c * P:(c + 1) * P],
                        rhs=mg[:, c * HID:(c + 1) * HID],
                        start=(c == 0),
                        stop=(c == cc - 1),
                    )
                agg_b = pagg.tile([P, HID], dt.bfloat16, tag="a2b", name="a2b")
                nc.scalar.activation(
                    out=agg_b[:], in_=agg_nm[:], func=AF.Copy, scale=1.0
                )
                aggsb = []
                for s in range(SH):
                    tp = psT.tile([P, P], dt.bfloat16, tag="tp", name="tp")
                    nc.tensor.transpose(
                        out=tp[:],
                        in_=agg_b[:, s * P:(s + 1) * P],
                        identity=ident_sb[:, :],
                    )
                    a = pagg.tile([P, P], f32r, tag="aggsb", name="aggsb")
                    nc.vector.tensor_copy(out=a[:], in_=tp[:])
                    aggsb.append(a)
                # dense2
                hps = psH.tile([P, OUT], dt.float32, tag="hps", name="hps")
                for s in range(SH):
                    nc.tensor.matmul(
                        out=hps[:],
                        lhsT=aggsb[s][:],
                        rhs=w2_sb[:, s * OUT:(s + 1) * OUT],
                        start=(s == 0),
                        stop=False,
                    )
                nc.tensor.matmul(
                    out=hps[:],
                    lhsT=sqrtin_sb[0:1, t * P:(t + 1) * P],
                    rhs=b2_sb[:],
                    start=False, stop=True,
                )
                h2 = phout.tile([P, OUT], dt.bfloat16, tag="h2", name="h2")
                nc.scalar.activation(
                    out=h2[:], in_=hps[:], func=AF.Lrelu,
                    scale=isi_sb[:, t:t + 1], alpha=0.01,
                )
                oh = psel.tile([P, G_PAD], dt.bfloat16, tag="oh", name="oh")
                nc.vector.tensor_tensor(
                    out=oh[:],
                    in0=iota_sb[:, :],
                    in1=gid_sb[:, t:t + 1].broadcast_to([P, G_PAD]),
                    op=OP.is_equal,
                )
                for s in range(SO):
                    nc.tensor.matmul(
                        out=hgps_list[s][:],
                        lhsT=h2[:, s * P:(s + 1) * P],
                        rhs=oh[:],
                        start=(t == 0),
                        stop=(t == T - 1),
                    )

            for t in range(T):
                layer2_tile(t)

            # ---- pooling finish + classifier -------------------------------
            hg_sb = pfin.tile([P, OUT], dt.float32)
            for s in range(SO):
                nc.vector.tensor_copy(
                    out=hg_sb[:, s * G_PAD:(s + 1) * G_PAD], in_=hgps_list[s][:]
                )
            nc.sync.dma_start(out=arin[:, :], in_=hg_sb[:])
            if fake_cc:
                nc.sync.dma_start(out=arout[:, :], in_=arin[:, :])
            else:
                nc.gpsimd.collective_compute(
                    "AllReduce",
                    OP.add,
                    replica_groups=rg,
                    ins=[arin[:, :]],
                    outs=[arout[:, :]],
                )
            hgr = pfin.tile([P, OUT], dt.float32)
            nc.sync.dma_start(out=hgr[:], in_=arout[:, :])
            ops = psH.tile([P, NCLS], dt.float32, tag="hps", name="ops")
            for s in range(SO):
                nc.tensor.matmul(
                    out=ops[:],
                    lhsT=hgr[:, s * G_PAD:(s + 1) * G_PAD],
                    rhs=wc_sb[:, s * NCLS:(s + 1) * NCLS],
                    start=(s == 0),
                    stop=False,
                )
            nc.tensor.matmul(
                out=ops[:], lhsT=cnt_sb[:], rhs=bc_sb[:], start=False, stop=True
            )
            res_sb = pfin.tile([P, NCLS], dt.float32)
            nc.scalar.activation(
                out=res_sb[:], in_=ops[:], func=AF.Copy, scale=rcnt_sb[:, 0:1]
            )
            nc.sync.dma_start(out=out_t[:, :], in_=res_sb[:])

    nc.compile()
    return nc


def _make_in_maps(pre, weights):
    W1, b1, W2, b2, Wc, bc = weights
    in_maps = []
    for c in range(NC):
        in_maps.append(
            {
                "scl1_sh": np.ascontiguousarray(pre["scl1_sh"][c]),
                "isi_sh": np.ascontiguousarray(pre["isi_sh"][c]),
                "sqrtin_sh": np.ascontiguousarray(pre["sqrtin_sh"][c]),
                "gid_sh": np.ascontiguousarray(pre["gid_sh"][c]),
                "idx_w": np.ascontiguousarray(pre["idx_wrapped"][c]),
                "dl_f": np.ascontiguousarray(pre["dl_f"][c]),
                "me1": np.ascontiguousarray(pre["me1"][c]),
                "W1": np.ascontiguousarray(W1, np.float32),
                "b1": np.ascontiguousarray(b1, np.float32).reshape(1, -1),
                "W2": np.ascontiguousarray(W2, np.float32),
                "b2": np.ascontiguousarray(b2, np.float32).reshape(1, -1),
                "Wc": np.ascontiguousarray(Wc, np.float32),
                "bc": np.ascontiguousarray(bc, np.float32).reshape(1, -1),
                "iota": pre["iota"],
                "ident": pre["ident"],
                "gcnt": pre["gcounts"].reshape(1, -1),
                "grcnt": (1.0 / pre["gcounts"]).reshape(-1, 1),
            }
        )
    return in_maps


def _run(nc, in_maps, trace=False):
    import time

    from concourse.bass_utils import run_bass_kernel_spmd

    last_exc = None
    for attempt in range(3):
        try:
            return run_bass_kernel_spmd(
                nc, in_maps, core_ids=list(range(NC)), trace=trace
            )
        except Exception as e:  # transient device wedges heal on retry
            last_exc = e
            time.sleep(20 * (attempt + 1))
    raise last_exc


def kernel(x, src, dst, graph_ids, W1, b1, W2, b2, Wc, bc, _trace=False,
           _return_results=False):
    x = np.asarray(x, dtype=np.float32)
    n_classes = int(np.asarray(Wc).shape[1])
    pre = _preprocess(x, src, dst, graph_ids, n_classes)
    hid = int(np.asarray(W1).shape[1])
    out_dim = int(np.asarray(W2).shape[1])
    nc = _build_program(pre, hid, out_dim, n_classes)
    weights = (np.asarray(W1), np.asarray(b1), np.asarray(W2),
               np.asarray(b2), np.asarray(Wc), np.asarray(bc))
    in_maps = _make_in_maps(pre, weights)
    res = _run(nc, in_maps, trace=_trace)
    out = res.results[0]["out"][:G_PAD, :n_classes].astype(np.float32)
    if _return_results:
        return out, res
    return out


# revision 12
# speedup vs baseline: 1.2244x; 1.0127x over previous
"""GraphConv classifier (DGL GraphConv x2 + mean-pool + linear) on 8 trn2 NeuronCores.

Strategy (sharding_hint: edge partitioning by dst + replicated weights +
all-reduce for pooling):
  - Nodes are degree-sorted and packed into 128-node dst tiles, row-sharded
    across the 8 cores (interleaved within each global rank block so all
    cores carry the same degree profile).
  - Layer 1: the per-edge source features (x * outdeg^-1/2, bf16) are
    expanded into an edge-ordered table on the HOST with one slot per
    (dst-tile chunk, dst position) such that chunk k holds every dst's k-th
    in-edge ("identity packing"). On device the aggregation is then a plain
    contiguous DMA stream + one DVE tensor_reduce per tile + one PE
    transpose. No gathers, no one-hot matmuls.
  - Layer 2: h1 is produced sharded, AllGather-ed (bf16 halves), then
    per-edge rows are dma_gather-ed (round-robin over the 4 SWDGE queues so
    descriptor generation runs on all four Q7 core pairs). Aggregation per
    chunk is ONE ldweights(one-hot S) + ONE wide matmul accumulating
    agg[dst, 256] in PSUM; the one-hot family for a whole tile is built in
    a single broadcast tensor_tensor is_equal.
  - Degree normalization is folded into per-partition activation scales and
    rank-1 bias matmuls; per-graph sums are AllReduce-d.
"""

import sys

for _p in ("/opt/trn_rl_repo", "/root/.axon_site/_ro/trn_rl_repo"):
    if _p not in sys.path:
        sys.path.append(_p)

import numpy as np
import ml_dtypes

NC = 8  # cores
P = 128  # partitions / tile node count
G_PAD = 128  # graphs (one tile)

_BF16 = ml_dtypes.bfloat16


def _preprocess(x, src, dst, graph_ids, n_classes):
    """Host-side preprocessing: degree-sorted node packing, layer-1
    edge-ordered feature table, layer-2 edge chunk packing, degree scales."""
    n, d1 = x.shape
    src = np.asarray(src, dtype=np.int64)
    dst = np.asarray(dst, dtype=np.int64)
    gid = np.asarray(graph_ids, dtype=np.int64)
    ne = len(src)

    T = -(-n // (P * NC))  # tiles per core (ceil)
    assert T % 2 == 0, "need an even tile count for the A/B half split"
    TH = T // 2
    npad = P * NC * T
    shard = P * T
    halfrows = NC * TH * P  # rows per gather table
    assert halfrows <= 32768, "int16 gather index range exceeded"

    outdeg = np.bincount(src, minlength=n).astype(np.float64)
    indeg = np.bincount(dst, minlength=n).astype(np.float64)
    iso = (1.0 / np.sqrt(np.maximum(outdeg, 1.0))).astype(np.float32)
    isi = (1.0 / np.sqrt(np.maximum(indeg, 1.0))).astype(np.float32)
    sqrtin = (np.sqrt(np.maximum(indeg, 1.0))).astype(np.float32)

    # --- degree-sorted node -> (core, tile, pos) packing --------------------
    # Global rank block r holds the r-th 1024 nodes by descending indegree;
    # core c takes every 8th node of the block, so every core's tile r has
    # the same degree profile (aligned chunk budgets, balanced load).
    order_nodes = np.argsort(-indeg, kind="stable")
    order_pad = np.full(npad, -1, np.int64)
    order_pad[:n] = order_nodes
    blocks = order_pad.reshape(T, NC * P)
    node_core = np.zeros(n, np.int64)
    node_tile = np.zeros(n, np.int64)
    node_pos = np.zeros(n, np.int64)
    perm = np.full(npad, -1, np.int64)  # slot -> original node (-1 pad)
    for t in range(T):
        blk = blocks[t]  # interleave: core c gets blk[c::NC]
        for c in range(NC):
            nodes = blk[c::NC]
            for p_i, v in enumerate(nodes):
                if v < 0:
                    continue
                node_core[v] = c
                node_tile[v] = t
                node_pos[v] = p_i
                perm[c * shard + t * P + p_i] = v

    # permuted per-node vectors [NC, shard]
    def gatherv(vec, fill):
        out = np.full(npad, fill, np.float32)
        m = perm >= 0
        out[m] = vec[perm[m]]
        return out.reshape(NC, shard)

    isi_sh = gatherv(isi, 1.0)
    sqrtin_sh = gatherv(sqrtin, 1.0)
    scl1_sh = gatherv(iso * isi, 1.0)
    gid_sh = gatherv(gid.astype(np.float32), -1.0).astype(_BF16)

    d_core = node_core[dst]
    d_tile = node_tile[dst]
    d_pos = node_pos[dst]

    # --- layer-1 identity-packed edge table ---------------------------------
    # per-tile chunk budget = max indegree among that tile's nodes (any core)
    CH1_t = np.zeros(T, np.int64)
    ideg_pad = np.zeros(npad, np.int64)
    m = perm >= 0
    ideg_pad[m] = indeg[perm[m]].astype(np.int64)
    CH1_t = np.maximum(
        ideg_pad.reshape(NC, T, P).max(axis=(0, 2)), 1
    )
    c1_off = np.zeros(T + 1, np.int64)
    c1_off[1:] = np.cumsum(CH1_t)
    ctot1 = int(c1_off[T])

    # rank of each edge within its dst node
    order_d = np.argsort(dst, kind="stable")
    ds = dst[order_d]
    grp = np.flatnonzero(np.r_[True, ds[1:] != ds[:-1]])
    st = np.zeros(ne, np.int64)
    st[grp] = grp
    st = np.maximum.accumulate(st)
    krank = np.arange(ne) - st  # in-edge rank within dst
    e_src = src[order_d]
    e_dc = d_core[order_d]
    e_dt = d_tile[order_d]
    e_dp = d_pos[order_d]
    chunk1 = c1_off[e_dt] + krank
    xt_rows = (np.asarray(x, np.float32) * iso[:, None]).astype(_BF16)
    me1_arr = np.zeros((NC, ctot1, P, d1), _BF16)
    me1_arr[e_dc, chunk1, e_dp] = xt_rows[e_src]
    # per-tile [P, d1, ch] layout (chunk innermost -> contiguous reduce reads)
    tile_blocks = []
    for t in range(T):
        blk = me1_arr[:, c1_off[t]:c1_off[t + 1]]  # [NC, ch, P, d1]
        tile_blocks.append(
            blk.transpose(0, 2, 3, 1).reshape(NC, P, -1)
        )
    me1 = np.ascontiguousarray(np.concatenate(tile_blocks, axis=2))

    # --- layer-2 edge packing (dense chunks + one-hot dl) -------------------
    s_half = (node_tile[src] >= TH).astype(np.int64)
    s_row = (
        node_core[src] * (TH * P)
        + (node_tile[src] - s_half * TH) * P
        + node_pos[src]
    )

    key = (d_core * T + d_tile) * 2 + s_half
    order = np.argsort(key, kind="stable")
    key_s = key[order]
    grp_start = np.flatnonzero(np.r_[True, key_s[1:] != key_s[:-1]])
    starts = np.zeros(len(key_s), dtype=np.int64)
    starts[grp_start] = grp_start
    starts = np.maximum.accumulate(starts)
    rnk = np.arange(len(key_s)) - starts

    counts = np.bincount(key, minlength=NC * T * 2)
    kA = counts[0::2].reshape(NC, T)
    kB = counts[1::2].reshape(NC, T)
    CA_t = np.maximum(1, -(-kA.max(axis=0) // P))  # [T]
    CB_t = np.maximum(1, -(-kB.max(axis=0) // P))
    C_t = CA_t + CB_t
    ctot = int(C_t.sum())

    c_off = np.zeros(T + 1, np.int64)
    c_off[1:] = np.cumsum(C_t)
    idx_arr = np.zeros((NC, ctot, P), dtype=np.int16)
    dl_arr = np.full((NC, ctot, P), -1.0, dtype=np.float32)

    e_core = d_core[order]
    e_tile = d_tile[order]
    e_half = s_half[order]
    e_srow = s_row[order]
    e_dpos = d_pos[order]
    chunk_i = (
        c_off[e_tile]
        + rnk // P
        + np.where(e_half == 1, CA_t[e_tile], 0)
    )
    part_i = rnk % P
    idx_arr[e_core, chunk_i, part_i] = e_srow.astype(np.int16)
    dl_arr[e_core, chunk_i, part_i] = e_dpos.astype(np.float32)

    def wrap_region(a):  # [NC, ck, P] -> [NC, 128, ck*8]
        nc_, ck, _ = a.shape
        flat = a.reshape(nc_, ck * P)
        plane = flat.reshape(nc_, ck * 8, 16).transpose(0, 2, 1)
        return np.tile(plane, (1, 8, 1))

    idx_cols = []
    icol_off = np.zeros(T + 1, np.int64)
    for t in range(T):
        regA = idx_arr[:, c_off[t]:c_off[t] + CA_t[t]]
        regB = idx_arr[:, c_off[t] + CA_t[t]:c_off[t + 1]]
        idx_cols.append(wrap_region(regA))
        idx_cols.append(wrap_region(regB))
        icol_off[t + 1] = icol_off[t] + 8 * (CA_t[t] + CB_t[t])
    idx_wrapped = np.ascontiguousarray(np.concatenate(idx_cols, axis=2))
    dl_f = np.ascontiguousarray(
        dl_arr.transpose(0, 2, 1).astype(_BF16)
    )  # [NC, P, ctot]

    gcounts = np.bincount(gid, minlength=G_PAD).astype(np.float32)
    gcounts = np.maximum(gcounts, 1.0)

    iota = np.broadcast_to(np.arange(P, dtype=np.float32), (P, P)).astype(_BF16)
    ident = np.eye(P, dtype=np.float32).astype(_BF16)

    return dict(
        n=n, d1=d1, T=T, TH=TH, npad=npad, shard=shard, halfrows=halfrows,
        CA_t=CA_t, CB_t=CB_t, C_t=C_t, ctot=ctot, c_off=c_off,
        icol_off=icol_off, CH1_t=CH1_t, c1_off=c1_off, ctot1=ctot1,
        isi_sh=isi_sh, sqrtin_sh=sqrtin_sh,
        scl1_sh=scl1_sh, gid_sh=gid_sh, idx_wrapped=idx_wrapped, dl_f=dl_f,
        me1=me1,
        gcounts=gcounts, iota=np.ascontiguousarray(iota),
        ident=np.ascontiguousarray(ident),
    )


def _build_program(pre, hid, out_dim, n_classes, timing_mode=False, fake_cc=None):
    import concourse.bacc as bacc
    import concourse.tile as tile
    import concourse.mybir as mybir

    dt = mybir.dt
    AF = mybir.ActivationFunctionType
    OP = mybir.AluOpType
    AX = mybir.AxisListType

    T, TH, D1, HID, OUT = pre["T"], pre["TH"], pre["d1"], hid, out_dim
    CA_t, CB_t, C_t = pre["CA_t"], pre["CB_t"], pre["C_t"]
    c_off, icol_off, ctot = pre["c_off"], pre["icol_off"], pre["ctot"]
    CH1_t, c1_off, ctot1 = pre["CH1_t"], pre["c1_off"], pre["ctot1"]
    SHARD, HROWS = pre["shard"], pre["halfrows"]
    HSH = SHARD // 2
    NCLS = n_classes
    CMAX = int(C_t.max())
    CH1MAX = int(CH1_t.max())
    assert D1 == P and HID % P == 0 and OUT % P == 0
    SH = HID // P  # hidden-dim slices
    SO = OUT // P

    if fake_cc is None:
        fake_cc = timing_mode
    nc = bacc.Bacc(
        "TRN2", target_bir_lowering=False, debug=False,
        num_devices=1 if timing_mode else NC,
        num_swdge_queues=4,
    )

    # ---- I/O ---------------------------------------------------------------
    def din(name, shape, dtype):
        return nc.dram_tensor(name, list(shape), dtype, kind="ExternalInput")

    scl1_in = din("scl1_sh", (SHARD,), dt.float32)
    isi_in = din("isi_sh", (SHARD,), dt.float32)
    sqrtin_in = din("sqrtin_sh", (SHARD,), dt.float32r)
    gid_in = din("gid_sh", (SHARD,), dt.bfloat16)
    idx_in = din("idx_w", (P, int(icol_off[T])), dt.int16)
    dl_in = din("dl_f", (P, ctot), dt.bfloat16)
    me1_in = din("me1", (P, ctot1 * D1), dt.bfloat16)
    w1_in = din("W1", (D1, HID), dt.float32r)
    b1_in = din("b1", (1, HID), dt.float32r)
    w2_in = din("W2", (HID, OUT), dt.float32r)
    b2_in = din("b2", (1, OUT), dt.float32r)
    wc_in = din("Wc", (OUT, NCLS), dt.float32)
    bc_in = din("bc", (1, NCLS), dt.float32)
    iota_in = din("iota", (P, P), dt.bfloat16)
    ident_in = din("ident", (P, P), dt.bfloat16)
    cnt_in = din("gcnt", (1, G_PAD), dt.float32)
    rcnt_in = din("grcnt", (G_PAD, 1), dt.float32)
    out_t = nc.dram_tensor(
        "out", [G_PAD, NCLS], dt.float32, kind="ExternalOutput"
    )

    h1loc = nc.dram_tensor("h1loc", [SHARD, HID], dt.bfloat16, kind="Internal")
    tbl2a = nc.dram_tensor(
        "tbl2a", [HROWS, HID], dt.bfloat16, kind="Internal", addr_space="Shared"
    )
    tbl2b = nc.dram_tensor(
        "tbl2b", [HROWS, HID], dt.bfloat16, kind="Internal", addr_space="Shared"
    )
    arin = nc.dram_tensor("arin", [P, OUT], dt.float32, kind="Internal")
    arout = nc.dram_tensor(
        "arout", [P, OUT], dt.float32, kind="Internal", addr_space="Shared"
    )

    rg = [list(range(NC))]
    f32r = dt.float32r

    def allgather(src_ap, dst_tensor):
        if fake_cc:
            nc.sync.dma_start(out=dst_tensor[0:HSH, :], in_=src_ap)
        else:
            nc.gpsimd.collective_compute(
                "AllGather",
                OP.bypass,
                replica_groups=rg,
                ins=[src_ap],
                outs=[dst_tensor[:, :]],
            )

    qctr = [0]  # round-robin SWDGE queue assignment

    with tile.TileContext(nc) as tc:
        with (
            tc.tile_pool(name="const", bufs=1) as cpool,
            tc.tile_pool(name="meta", bufs=4) as pmeta,
            tc.tile_pool(name="mg", bufs=5) as pmg,
            tc.tile_pool(name="mg1", bufs=4) as pmg1,
            tc.tile_pool(name="sel", bufs=3) as psel,
            tc.tile_pool(name="aggs", bufs=3) as pagg,
            tc.tile_pool(name="hout", bufs=3) as phout,
            tc.tile_pool(name="fin", bufs=1) as pfin,
            tc.tile_pool(name="psA", bufs=2, space="PSUM") as psA,
            tc.tile_pool(name="psT", bufs=2, space="PSUM") as psT,
            tc.tile_pool(name="psH", bufs=2, space="PSUM") as psH,
            tc.tile_pool(name="psG", bufs=1, space="PSUM") as psG,
        ):
            # ---- constants -------------------------------------------------
            iota_sb = cpool.tile([P, P], dt.bfloat16)
            nc.sync.dma_start(out=iota_sb[:], in_=iota_in[:, :])
            ident_sb = cpool.tile([P, P], dt.bfloat16)
            nc.sync.dma_start(out=ident_sb[:], in_=ident_in[:, :])
            scl1_sb = cpool.tile([P, T], dt.float32)
            nc.sync.dma_start(
                out=scl1_sb[:], in_=scl1_in.ap().rearrange("(t p) -> p t", p=P)
            )
            isi_sb = cpool.tile([P, T], dt.float32)
            nc.sync.dma_start(
                out=isi_sb[:], in_=isi_in.ap().rearrange("(t p) -> p t", p=P)
            )
            gid_sb = cpool.tile([P, T], dt.bfloat16)
            nc.sync.dma_start(
                out=gid_sb[:], in_=gid_in.ap().rearrange("(t p) -> p t", p=P)
            )
            sqrtin_sb = cpool.tile([1, SHARD], f32r)
            nc.sync.dma_start(out=sqrtin_sb[:], in_=sqrtin_in.ap()[None, :])
            w1_sb = cpool.tile([P, HID], f32r)
            nc.sync.dma_start(out=w1_sb[:], in_=w1_in[:, :])
            b1_sb = cpool.tile([1, HID], f32r)
            nc.sync.dma_start(out=b1_sb[:], in_=b1_in[:, :])
            w2_sb = cpool.tile([P, SH * OUT], f32r)  # [128, slice, OUT]
            nc.sync.dma_start(
                out=w2_sb[:],
                in_=w2_in.ap().rearrange("(s k) o -> k s o", k=P),
            )
            b2_sb = cpool.tile([1, OUT], f32r)
            nc.sync.dma_start(out=b2_sb[:], in_=b2_in[:, :])
            wc_sb = cpool.tile([P, SO * NCLS], dt.float32)
            nc.sync.dma_start(
                out=wc_sb[:],
                in_=wc_in.ap().rearrange("(s k) o -> k s o", k=P),
            )
            bc_sb = cpool.tile([1, NCLS], dt.float32)
            nc.sync.dma_start(out=bc_sb[:], in_=bc_in[:, :])
            cnt_sb = cpool.tile([1, G_PAD], dt.float32)
            nc.sync.dma_start(out=cnt_sb[:], in_=cnt_in[:, :])
            rcnt_sb = cpool.tile([G_PAD, 1], dt.float32)
            nc.sync.dma_start(out=rcnt_sb[:], in_=rcnt_in[:, :])
            idx_all = cpool.tile([P, int(icol_off[T])], dt.int16)
            nc.sync.dma_start(out=idx_all[:], in_=idx_in[:, :])
            dl_all = cpool.tile([P, ctot], dt.bfloat16)
            nc.sync.dma_start(out=dl_all[:], in_=dl_in[:, :])

            # ---- phase 1: streamed identity-packed aggregation -------------
            def layer1_tile(t):
                ch = int(CH1_t[t])
                co = int(c1_off[t])
                mg = pmg1.tile(
                    [P, CH1MAX * D1], dt.bfloat16, tag="mg1", name="mg1"
                )
                nc.sync.dma_start(
                    out=mg[:, : ch * D1],
                    in_=me1_in[:, co * D1:(co + ch) * D1],
                )
                agg_f = pagg.tile([P, D1], dt.float32, tag="aggf", name="aggf")
                nc.vector.tensor_reduce(
                    out=agg_f[:, :],
                    in_=mg[:, : ch * D1].rearrange("p (d c) -> p d c", c=ch),
                    axis=AX.X,
                    op=OP.add,
                )
                agg_b = pagg.tile([P, D1], dt.bfloat16, tag="aggb", name="aggb")
                nc.scalar.activation(
                    out=agg_b[:], in_=agg_f[:], func=AF.Copy, scale=1.0
                )
                tp = psT.tile([P, P], dt.bfloat16, tag="tp", name="tp")
                nc.tensor.transpose(
                    out=tp[:], in_=agg_b[:], identity=ident_sb[:, :]
                )
                aggsb = pagg.tile([P, P], f32r, tag="aggsb", name="aggsb")
                nc.scalar.activation(
                    out=aggsb[:], in_=tp[:], func=AF.Copy, scale=1.0
                )
                # dense1
                hps = psH.tile([P, HID], dt.float32, tag="hps", name="hps")
                nc.tensor.matmul(
                    out=hps[:], lhsT=aggsb[:], rhs=w1_sb[:],
                    start=True, stop=False,
                )
                nc.tensor.matmul(
                    out=hps[:],
                    lhsT=sqrtin_sb[0:1, t * P:(t + 1) * P],
                    rhs=b1_sb[:],
                    start=False, stop=True,
                )
                h1s = phout.tile([P, HID], dt.bfloat16, tag="h1s", name="h1s")
                nc.scalar.activation(
                    out=h1s[:], in_=hps[:], func=AF.Lrelu,
                    scale=scl1_sb[:, t:t + 1], alpha=0.01,
                )
                nc.sync.dma_start(out=h1loc[t * P:(t + 1) * P, :], in_=h1s[:])

            for t in range(T):
                layer1_tile(t)
                if t == TH - 1:
                    allgather(h1loc[0:HSH, :], tbl2a)
            allgather(h1loc[HSH:SHARD, :], tbl2b)

            # ---- phase 2: gathered one-hot aggregation + pooling -----------
            hgps_list = [
                psG.tile([P, G_PAD], dt.float32, name=f"hgps{s}")
                for s in range(SO)
            ]

            def layer2_tile(t):
                ca, cb = int(CA_t[t]), int(CB_t[t])
                cc = ca + cb
                io, co = int(icol_off[t]), int(c_off[t])
                idx_sb = idx_all[:, io:io + cc * 8]
                dl_sb = dl_all[:, co:co + cc]
                mg = pmg.tile([P, CMAX * HID], dt.bfloat16, tag="mg", name="mg")
                # A/B half-table gathers round-robin across SWDGE queues
                for tbl, lo, hi in ((tbl2a, 0, ca), (tbl2b, ca, cc)):
                    nc.gpsimd.dma_gather(
                        out_ap=mg[:, lo * HID:hi * HID].rearrange(
                            "p (c e) -> p c e", e=HID
                        ),
                        in_ap=tbl[0:HROWS, :],
                        idxs_ap=idx_all[:, io + lo * 8:io + hi * 8],
                        num_idxs=(hi - lo) * P,
                        num_idxs_reg=(hi - lo) * P,
                        elem_size=HID,
                        single_packet=False,
                        queue_num=qctr[0] % 4,
                    )
                    qctr[0] += 1
                s_all = psel.tile([P, CMAX * P], dt.bfloat16, tag="S", name="S")
                nc.vector.tensor_tensor(
                    out=s_all[:, : cc * P].rearrange("p (c j) -> p c j", c=cc),
                    in0=iota_sb[:, :].unsqueeze(1).broadcast_to([P, cc, P]),
                    in1=dl_all[:, co:co + cc].unsqueeze(2).broadcast_to(
                        [P, cc, P]
                    ),
                    op=OP.is_equal,
                )
                agg_nm = psA.tile(
                    [P, HID], dt.float32, tag="aggnm", name="aggnm"
                )
                for c in range(cc):
                    nc.tensor.matmul(
                        out=agg_nm[:],
                        lhsT=s_all[:, c * P:(c + 1) * P],
                        rhs=mg[:, c * HID:(c + 1) * HID],
                        start=(c == 0),
                        stop=(c == cc - 1),
                    )
                agg_b = pagg.tile([P, HID], dt.bfloat16, tag="a2b", name="a2b")
                nc.scalar.activation(
                    out=agg_b[:], in_=agg_nm[:], func=AF.Copy, scale=1.0
                )
                aggsb = []
                for s in range(SH):
                    tp = psT.tile([P, P], dt.bfloat16, tag="tp", name="tp")
                    nc.tensor.transpose(
                        out=tp[:],
                        in_=agg_b[:, s * P:(s + 1) * P],
                        identity=ident_sb[:, :],
                    )
                    a = pagg.tile([P, P], f32r, tag="aggsb", name="aggsb")
                    nc.vector.tensor_copy(out=a[:], in_=tp[:])
                    aggsb.append(a)
                # dense2
                hps = psH.tile([P, OUT], dt.float32, tag="hps", name="hps")
                for s in range(SH):
                    nc.tensor.matmul(
                        out=hps[:],
                        lhsT=aggsb[s][:],
                        rhs=w2_sb[:, s * OUT:(s + 1) * OUT],
                        start=(s == 0),
                        stop=False,
                    )
                nc.tensor.matmul(
                    out=hps[:],
                    lhsT=sqrtin_sb[0:1, t * P:(t + 1) * P],
                    rhs=b2_sb[:],
                    start=False, stop=True,
                )
                h2 = phout.tile([P, OUT], dt.bfloat16, tag="h2", name="h2")
                nc.scalar.activation(
                    out=h2[:], in_=hps[:], func=AF.Lrelu,
                    scale=isi_sb[:, t:t + 1], alpha=0.01,
                )
                oh = psel.tile([P, G_PAD], dt.bfloat16, tag="oh", name="oh")
                nc.vector.tensor_tensor(
                    out=oh[:],
                    in0=iota_sb[:, :],
                    in1=gid_sb[:, t:t + 1].broadcast_to([P, G_PAD]),
                    op=OP.is_equal,
                )
                for s in range(SO):
                    nc.tensor.matmul(
                        out=hgps_list[s][:],
                        lhsT=h2[:, s * P:(s + 1) * P],
                        rhs=oh[:],
                        start=(t == 0),
                        stop=(t == T - 1),
                    )

            for t in range(T):
                layer2_tile(t)

            # ---- pooling finish + classifier -------------------------------
            hg_sb = pfin.tile([P, OUT], dt.float32)
            for s in range(SO):
                nc.vector.tensor_copy(
                    out=hg_sb[:, s * G_PAD:(s + 1) * G_PAD], in_=hgps_list[s][:]
                )
            nc.sync.dma_start(out=arin[:, :], in_=hg_sb[:])
            if fake_cc:
                nc.sync.dma_start(out=arout[:, :], in_=arin[:, :])
            else:
                nc.gpsimd.collective_compute(
                    "AllReduce",
                    OP.add,
                    replica_groups=rg,
                    ins=[arin[:, :]],
                    outs=[arout[:, :]],
                )
            hgr = pfin.tile([P, OUT], dt.float32)
            nc.sync.dma_start(out=hgr[:], in_=arout[:, :])
            ops = psH.tile([P, NCLS], dt.float32, tag="hps", name="ops")
            for s in range(SO):
                nc.tensor.matmul(
                    out=ops[:],
                    lhsT=hgr[:, s * G_PAD:(s + 1) * G_PAD],
                    rhs=wc_sb[:, s * NCLS:(s + 1) * NCLS],
                    start=(s == 0),
                    stop=False,
                )
            nc.tensor.matmul(
                out=ops[:], lhsT=cnt_sb[:], rhs=bc_sb[:], start=False, stop=True
            )
            res_sb = pfin.tile([P, NCLS], dt.float32)
            nc.scalar.activation(
                out=res_sb[:], in_=ops[:], func=AF.Copy, scale=rcnt_sb[:, 0:1]
            )
            nc.sync.dma_start(out=out_t[:, :], in_=res_sb[:])

    nc.compile()
    return nc


def _make_in_maps(pre, weights):
    W1, b1, W2, b2, Wc, bc = weights
    in_maps = []
    for c in range(NC):
        in_maps.append(
            {
                "scl1_sh": np.ascontiguousarray(pre["scl1_sh"][c]),
                "isi_sh": np.ascontiguousarray(pre["isi_sh"][c]),
                "sqrtin_sh": np.ascontiguousarray(pre["sqrtin_sh"][c]),
                "gid_sh": np.ascontiguousarray(pre["gid_sh"][c]),
                "idx_w": np.ascontiguousarray(pre["idx_wrapped"][c]),
                "dl_f": np.ascontiguousarray(pre["dl_f"][c]),
                "me1": np.ascontiguousarray(pre["me1"][c]),
                "W1": np.ascontiguousarray(W1, np.float32),
                "b1": np.ascontiguousarray(b1, np.float32).reshape(1, -1),
                "W2": np.ascontiguousarray(W2, np.float32),
                "b2": np.ascontiguousarray(b2, np.float32).reshape(1, -1),
                "Wc": np.ascontiguousarray(Wc, np.float32),
                "bc": np.ascontiguousarray(bc, np.float32).reshape(1, -1),
                "iota": pre["iota"],
                "ident": pre["ident"],
                "gcnt": pre["gcounts"].reshape(1, -1),
                "grcnt": (1.0 / pre["gcounts"]).reshape(-1, 1),
            }
        )
    return in_maps


def _run(nc, in_maps, trace=False):
    import time

    from concourse.bass_utils import run_bass_kernel_spmd

    last_exc = None
    for attempt in range(3):
        try:
            return run_bass_kernel_spmd(
                nc, in_maps, core_ids=list(range(NC)), trace=trace
            )
        except Exception as e:  # transient device wedges heal on retry
            last_exc = e
            time.sleep(20 * (attempt + 1))
    raise last_exc


def kernel(x, src, dst, graph_ids, W1, b1, W2, b2, Wc, bc, _trace=False,
           _return_results=False):
    x = np.asarray(x, dtype=np.float32)
    n_classes = int(np.asarray(Wc).shape[1])
    pre = _preprocess(x, src, dst, graph_ids, n_classes)
    hid = int(np.asarray(W1).shape[1])
    out_dim = int(np.asarray(W2).shape[1])
    nc = _build_program(pre, hid, out_dim, n_classes)
    weights = (np.asarray(W1), np.asarray(b1), np.asarray(W2),
               np.asarray(b2), np.asarray(Wc), np.asarray(bc))
    in_maps = _make_in_maps(pre, weights)
    res = _run(nc, in_maps, trace=_trace)
    out = res.results[0]["out"][:G_PAD, :n_classes].astype(np.float32)
    if _return_results:
        return out, res
    return out
